# revision 24
# baseline (speedup 1.0000x reference)
"""Neural CDE (RK4 scan over a tiny MLP vector field) on 8 TRN2 cores.

Strategy: pure batch data-parallelism (1024 batch -> 128 per core). State is
kept transposed hT [H=32, B=128] (batch on the free axis) so every matmul
contracts over the small feature dims on PE partitions. The einsum
g = f(h).dX, the RK4 state updates h + c*k, and the next substep's first
matmul are all fused into PE PSUM-accumulated matmuls:

    z1(s+1) = W1aug^T hTaug (pre-issued base) + c_s (R W1)^T u_s
    h_next  = I^T h + sum_s (w_s/6) R^T u_s          (R = tiled identity)

where u_s = tanh(W4^T z3 + b4) * broadcast(dX). Biases are folded into the
matmuls via an augmented ones-row, so the inter-layer nonlinearities are pure
max(x, 0) on DVE and one tanh on ACT per substep.

The critical path per substep is 9 engine ops:
  red-mms(PE) -> relu1(DVE) -> MM2(PE) -> relu2 -> MM3(PE) -> relu3
  -> MM4A/B(PE) -> tanh(ACT) -> mult(DVE)
"""

import numpy as np
from contextlib import ExitStack

import concourse.bass as bass
import concourse.tile as tile
from concourse import bacc, mybir
from concourse.bass_utils import run_bass_kernel_spmd

B, T, D, H, HH = 1024, 1000, 6, 32, 15
NCORES = 8
P = B // NCORES          # 128 batch per core
TS_FULL = T - 1          # 999 scan steps
CH = 32                  # dx chunk size (steps per DMA)

F32 = mybir.dt.float32
R32 = mybir.dt.float32r
H16 = mybir.dt.float16
TANH = mybir.ActivationFunctionType.Tanh
USE_F32R = True
# The W1^T h base path stays float32r (single-pass PE mode, ~11-bit mantissa);
# the MLP path (layers 2-4 and the u reductions) runs fp16, which matches
# f32r's mantissa but moves 1 column/cycle on the PE instead of 1/4.
MM_DT = R32 if USE_F32R else F32


def _r(ap):
    """Matmul operand passthrough (tiles already carry the matmul dtype)."""
    return ap


def _rd(ap):
    """Bitcast an fp32 DRAM AP to float32r for DMA into an f32r tile."""
    return ap.bitcast(R32) if USE_F32R else ap


def _emit(ctx, tc, ins, out_ap, ts):
    nc = tc.nc
    nchunk = (ts + CH - 1) // CH
    sb = ctx.enter_context(tc.tile_pool(name="sb", bufs=1))
    ps = ctx.enter_context(tc.tile_pool(name="ps", bufs=1, space="PSUM"))

    # ---- persistent SBUF tiles ----
    w1 = sb.tile([H + 1, HH], MM_DT, name="w1")
    w2 = sb.tile([HH + 1, HH], H16, name="w2")
    w3 = sb.tile([HH + 1, HH], H16, name="w3")
    w4a = sb.tile([HH + 1, 96], H16, name="w4a")
    w4b = sb.tile([HH + 1, 96], H16, name="w4b")
    rw1_h = sb.tile([96, HH], H16, name="rw1_h")   # 0.5 * tile(W1)
    rw1_1 = sb.tile([96, HH], H16, name="rw1_1")   # 1.0 *
    rw1_6 = sb.tile([96, HH], H16, name="rw1_6")   # 1/6 *
    rw1_3 = sb.tile([96, HH], H16, name="rw1_3")   # 1/3 *
    rsel_6 = sb.tile([96, H], H16, name="rsel_6")  # 1/6 * tile(I)
    rsel_3 = sb.tile([96, H], H16, name="rsel_3")  # 1/3 *
    sel_a = sb.tile([D, 96], MM_DT, name="sel_a")
    sel_b = sb.tile([D, 96], MM_DT, name="sel_b")
    # h kept two ways: exact fp32 accumulator (DVE adds only, never rounded
    # by the PE path) and an f32r copy feeding the W1^T h base matmuls.
    h_ex = sb.tile([H, P], F32, name="h_ex")
    h_mm = sb.tile([H + 1, P], MM_DT, name="h_mm")
    z1 = sb.tile([HH + 1, P], H16, name="z1")
    z2 = sb.tile([HH + 1, P], H16, name="z2")
    z3 = sb.tile([HH + 1, P], H16, name="z3")
    tt = sb.tile([96, 2 * P], F32, name="tt")
    uu_a = sb.tile([96, 2 * P], H16, name="uu_a")
    uu_b = sb.tile([96, 2 * P], H16, name="uu_b")
    bc = [sb.tile([96, 2 * P], F32, name=f"bc{i}") for i in range(2)]
    chunk = [sb.tile([D, CH * P], MM_DT, name=f"chunk{i}") for i in range(2)]

    # ---- PSUM tiles (8 banks exactly) ----
    pz1 = [ps.tile([HH, P], F32, name=f"pz1_{s}") for s in range(4)]
    pz23 = ps.tile([HH, 2 * P], F32, name="pz23")
    pf = ps.tile([96, 2 * P], F32, name="pf")
    ph = ps.tile([H, P], F32, name="ph")
    pbc = ps.tile([96, 2 * P], F32, name="pbc")

    # ---- one-time loads ----
    for t_sb, name in [(w1, "w1"), (sel_a, "sel_a"), (sel_b, "sel_b")]:
        nc.sync.dma_start(out=t_sb[:, :], in_=_rd(ins[name][:, :]))
    for t_sb, name in [
        (w2, "w2"), (w3, "w3"), (w4a, "w4a"), (w4b, "w4b"),
        (rw1_h, "rw1_h"), (rw1_1, "rw1_1"), (rw1_6, "rw1_6"), (rw1_3, "rw1_3"),
        (rsel_6, "rsel_6"), (rsel_3, "rsel_3"),
    ]:
        nc.sync.dma_start(out=t_sb[:, :], in_=ins[name][:, :])
    nc.sync.dma_start(out=h_mm[:, :], in_=_rd(ins["h0t"][:, :]))
    nc.sync.dma_start(out=h_ex[:, :], in_=ins["h0t"][0:H, :])
    nc.sync.dma_start(out=chunk[0][:, :], in_=_rd(ins["dxc"][0, :, :]))
    if nchunk > 1:
        nc.sync.dma_start(out=chunk[1][:, :], in_=_rd(ins["dxc"][1, :, :]))
    nc.sync.dma_start(out=z1[HH:HH + 1, :], in_=ins["ones16"][:, :])
    nc.sync.dma_start(out=z2[HH:HH + 1, :], in_=ins["ones16"][:, :])
    nc.sync.dma_start(out=z3[HH:HH + 1, :], in_=ins["ones16"][:, :])

    # bcast tiles for t=0
    nc.tensor.matmul(pbc[:, 0:P], lhsT=_r(sel_a[:, :]), rhs=_r(chunk[0][:, 0:P]),
                     start=True, stop=True)
    nc.tensor.matmul(pbc[:, P:2 * P], lhsT=_r(sel_b[:, :]), rhs=_r(chunk[0][:, 0:P]),
                     start=True, stop=True)
    nc.vector.tensor_copy(bc[0][:, :], pbc[:, :])
    # substep-0 preactivation for t=0 (no red contributions yet)
    nc.tensor.matmul(pz1[0][:, :], lhsT=_r(w1[:, :]), rhs=_r(h_mm[:, :]),
                     start=True, stop=True)

    uu2 = [uu_a, uu_b]                       # u of global substep g in uu2[g%2]
    C_SUB = [rw1_h, rw1_h, rw1_1]            # scale for h + c*k inputs
    W_RW1 = [rw1_6, rw1_3, rw1_3, rw1_6]     # RK4 combine weights into z1s0'
    W_RSEL = [rsel_6, rsel_3, rsel_3, rsel_6]

    def red_half(dst, lhs, u, half, start=False, stop=False):
        sl = slice(0, P) if half == 0 else slice(P, 2 * P)
        nc.tensor.matmul(dst, lhsT=_r(lhs[:, :]), rhs=_r(u[:, sl]),
                         start=start, stop=stop, skip_group_check=True)

    def base_mm(dst, lhs, rhs):
        nc.tensor.matmul(dst, lhsT=_r(lhs), rhs=_r(rhs),
                         start=True, stop=False, skip_group_check=True)

    for t in range(ts):
        last = t == ts - 1
        bct = bc[t % 2]
        # dx chunk prefetch (chunks 0,1 preloaded before the loop)
        ci = t // CH + 1
        if t % CH == 0 and 2 <= ci < nchunk:
            nc.sync.dma_start(out=chunk[ci % 2][:, :],
                              in_=_rd(ins["dxc"][ci, :, :]))

        for s in range(4):
            g = t * 4 + s
            u_cur = uu2[g % 2]
            # relu of layer-1 preactivation (bias folded into the matmuls);
            # the reductions producing pz1[s] were emitted at the end of the
            # previous substep, ahead of everything below in the PE queue.
            nc.vector.tensor_scalar_max(z1[0:HH, :], pz1[s][:, :], 0.0)
            if s == 3 and not last:
                # dx bcast for step t+1 (pbc was filled by the s==2 PE work);
                # emitted after relu1 so it runs in the relu1->relu2 DVE gap
                nc.vector.tensor_copy(bc[(t + 1) % 2][:, :], pbc[:, :])
            if s == 0 and t > 0:
                # h(t) = h(t-1) + RK4 increment: exact fp32 add on DVE (the
                # PE never touches h_ex, so h carries no f32r rounding), plus
                # a rounded copy for the W1^T h base matmuls. Emitted after
                # relu1 so both run in the relu1->relu2 DVE gap.
                nc.vector.tensor_add(h_ex[:, :], h_ex[:, :], ph[:, :])
                nc.vector.tensor_copy(h_mm[0:H, :], h_ex[:, :])
            nc.tensor.matmul(pz23[:, 0:P], lhsT=_r(w2[:, :]), rhs=_r(z1[:, :]),
                             start=True, stop=True, skip_group_check=True)
            nc.vector.tensor_scalar_max(z2[0:HH, :], pz23[:, 0:P], 0.0)
            nc.tensor.matmul(pz23[:, P:2 * P], lhsT=_r(w3[:, :]), rhs=_r(z2[:, :]),
                             start=True, stop=True, skip_group_check=True)
            nc.vector.tensor_scalar_max(z3[0:HH, :], pz23[:, P:2 * P], 0.0)
            nc.tensor.matmul(pf[:, 0:P], lhsT=_r(w4a[:, :]), rhs=_r(z3[:, :]),
                             start=True, stop=True, skip_group_check=True)
            nc.tensor.matmul(pf[:, P:2 * P], lhsT=_r(w4b[:, :]), rhs=_r(z3[:, :]),
                             start=True, stop=True, skip_group_check=True)
            # ---- PE fill work (u-independent): base matmuls + dx bcast ----
            if s == 0:
                # all four W1^T h bases for this step (pz1[k] banks were
                # consumed by relu1 of substep k in earlier queue positions)
                base_mm(pz1[1][:, :], w1[:, :], h_mm[:, :])
                base_mm(pz1[2][:, :], w1[:, :], h_mm[:, :])
                base_mm(pz1[3][:, :], w1[:, :], h_mm[:, :])
                if not last:
                    base_mm(pz1[0][:, :], w1[:, :], h_mm[:, :])
            elif s == 2 and not last:
                tn = t + 1
                sl = slice((tn % CH) * P, (tn % CH) * P + P)
                cn = chunk[(tn // CH) % 2]
                nc.tensor.matmul(pbc[:, 0:P], lhsT=_r(sel_a[:, :]),
                                 rhs=_r(cn[:, sl]), start=True, stop=True,
                                 skip_group_check=True)
                nc.tensor.matmul(pbc[:, P:2 * P], lhsT=_r(sel_b[:, :]),
                                 rhs=_r(cn[:, sl]), start=True, stop=True,
                                 skip_group_check=True)
            # ---- tail: tanh/mult pipelined in a/b halves ----
            nc.scalar.activation(tt[:, 0:P], pf[:, 0:P], TANH)
            nc.scalar.activation(tt[:, P:2 * P], pf[:, P:2 * P], TANH)
            nc.vector.tensor_mul(u_cur[:, 0:P], tt[:, 0:P], bct[:, 0:P])
            nc.vector.tensor_mul(u_cur[:, P:2 * P], tt[:, P:2 * P],
                                 bct[:, P:2 * P])
            # ---- all u_cur reductions; on-path dst first, a half first ----
            if s < 3:
                red_half(pz1[s + 1][:, :], C_SUB[s], u_cur, 0)
                red_half(pz1[s + 1][:, :], C_SUB[s], u_cur, 1, stop=True)
                if not last:
                    red_half(pz1[0][:, :], W_RW1[s], u_cur, 0)
                    red_half(pz1[0][:, :], W_RW1[s], u_cur, 1)
                red_half(ph[:, :], W_RSEL[s], u_cur, 0, start=(s == 0))
                red_half(ph[:, :], W_RSEL[s], u_cur, 1)
            else:
                if not last:
                    red_half(pz1[0][:, :], W_RW1[3], u_cur, 0)
                    red_half(pz1[0][:, :], W_RW1[3], u_cur, 1, stop=True)
                red_half(ph[:, :], W_RSEL[3], u_cur, 0)
                red_half(ph[:, :], W_RSEL[3], u_cur, 1, stop=True)

    nc.vector.tensor_add(h_ex[:, :], h_ex[:, :], ph[:, :])
    nc.sync.dma_start(out=out_ap[:, :], in_=h_ex[:, :])


_CACHE = {}


def _input_specs(ts):
    nchunk = (ts + CH - 1) // CH
    return {
        "w1": ((H + 1, HH), F32), "w2": ((HH + 1, HH), H16),
        "w3": ((HH + 1, HH), H16),
        "w4a": ((HH + 1, 96), H16), "w4b": ((HH + 1, 96), H16),
        "rw1_h": ((96, HH), H16), "rw1_1": ((96, HH), H16),
        "rw1_6": ((96, HH), H16), "rw1_3": ((96, HH), H16),
        "rsel_6": ((96, H), H16), "rsel_3": ((96, H), H16),
        "sel_a": ((D, 96), F32), "sel_b": ((D, 96), F32),
        "h0t": ((H + 1, P), F32), "dxc": ((nchunk, D, CH * P), F32),
        "ones16": ((1, P), H16),
    }


def build(ts=TS_FULL):
    if ts in _CACHE:
        return _CACHE[ts]
    nc = bacc.Bacc("TRN2", target_bir_lowering=False, debug=False,
                   enable_asserts=False, num_devices=NCORES)
    ins = {
        name: nc.dram_tensor(name, list(shape), dt, kind="ExternalInput").ap()
        for name, (shape, dt) in _input_specs(ts).items()
    }
    out_ap = nc.dram_tensor("ht_out", [H, P], F32, kind="ExternalOutput").ap()
    with tile.TileContext(nc, trace_sim=False) as tc:
        with ExitStack() as ctx:
            _emit(ctx, tc, ins, out_ap, ts)
    nc.compile()
    _CACHE[ts] = nc
    return nc


def host_prep(coeffs, W0, b0, W1, b1, W2, b2, W3, b3, W4, b4, ts=TS_FULL):
    f32 = np.float32
    coeffs = np.ascontiguousarray(coeffs, dtype=f32)
    h0 = coeffs[:, 0, :] @ W0.astype(f32) + b0.astype(f32)      # [B, H]
    dX = coeffs[:, 1:ts + 1, :] - coeffs[:, :ts, :]             # [B, ts, D]

    W1 = W1.astype(f32)
    W4r = W4.astype(f32).reshape(HH, H, D)
    W4P = W4r.transpose(0, 2, 1).reshape(HH, D * H)             # cols d*32+i
    b4P = b4.astype(f32).reshape(H, D).T.reshape(D * H)
    RW1 = np.tile(W1, (3, 1)).astype(f32)                       # [96, HH]
    Rsel = np.tile(np.eye(H, dtype=f32), (3, 1))                # [96, H]
    sel_a = np.zeros((D, 96), f32)
    sel_b = np.zeros((D, 96), f32)
    for d in range(3):
        sel_a[d, 32 * d:32 * d + 32] = 1.0
        sel_b[d + 3, 32 * d:32 * d + 32] = 1.0

    f16 = np.float16
    shared_f32 = {
        "w1": np.concatenate([W1, b1.astype(f32)[None]], 0),
        "sel_a": sel_a, "sel_b": sel_b,
    }
    shared_f16 = {
        "w2": np.concatenate([W2.astype(f32), b2.astype(f32)[None]], 0),
        "w3": np.concatenate([W3.astype(f32), b3.astype(f32)[None]], 0),
        "w4a": np.concatenate([W4P[:, :96], b4P[None, :96]], 0),
        "w4b": np.concatenate([W4P[:, 96:], b4P[None, 96:]], 0),
        "rw1_h": (0.5 * RW1), "rw1_1": RW1,
        "rw1_6": RW1 / 6.0, "rw1_3": RW1 / 3.0,
        "rsel_6": Rsel / 6.0, "rsel_3": Rsel / 3.0,
        "ones16": np.ones((1, P), f16),
    }
    shared = {k: np.ascontiguousarray(v, dtype=f32) for k, v in shared_f32.items()}
    shared.update(
        {k: np.ascontiguousarray(v, dtype=f16) for k, v in shared_f16.items()})

    nchunk = (ts + CH - 1) // CH
    in_maps = []
    for c in range(NCORES):
        sl = slice(c * P, (c + 1) * P)
        h0t = np.concatenate([h0[sl].T, np.ones((1, P), f32)], 0)
        dxt = dX[sl].transpose(1, 2, 0)                          # [ts, D, P]
        pad = np.zeros((nchunk * CH, D, P), f32)
        pad[:ts] = dxt
        dxc = pad.reshape(nchunk, CH, D, P).transpose(0, 2, 1, 3).reshape(
            nchunk, D, CH * P)
        m = dict(shared)
        m["h0t"] = np.ascontiguousarray(h0t, f32)
        m["dxc"] = np.ascontiguousarray(dxc, f32)
        in_maps.append(m)
    return in_maps


def run_device(in_maps, ts=TS_FULL, **kw):
    nc = build(ts)
    return run_bass_kernel_spmd(nc, in_maps, list(range(NCORES)), **kw)


def kernel(coeffs, W0, b0, W1, b1, W2, b2, W3, b3, W4, b4, Wf, bf):
    in_maps = host_prep(coeffs, W0, b0, W1, b1, W2, b2, W3, b3, W4, b4)
    res = run_device(in_maps)
    hT = np.stack([res.results[c]["ht_out"] for c in range(NCORES)])  # [8,H,P]
    h_all = hT.transpose(0, 2, 1).reshape(B, H)
    return (h_all @ Wf.astype(np.float32) + bf.astype(np.float32)).astype(
        np.float32)

